# revision 33
# baseline (speedup 1.0000x reference)
"""Trainium2 Bass kernel: 3x3 stride-1 pad-1 conv2d, NCHW int32 (quantized).

Contract: kernel(x, weight) takes the FULL inputs
  x      (32, 256, 56, 56) int32, values in [0, 16)
  weight (256, 256, 3, 3)  int32, values in [0, 15)
and returns the FULL (32, 256, 56, 56) int32 output of
conv2d(stride=1, padding=1), bit-exact.

Strategy
--------
Data-parallel over batch: 32 images -> 8 NeuronCores x 4 images, weights
replicated. Inputs are small non-negative ints, exactly representable in fp8
e4m3; fp8 products accumulate exactly in fp32 PSUM (max accumulator
15*14*9*256 < 2^24), so the whole computation is exact integer arithmetic.
The conv runs as 9 shifted matmuls per output tile with DoubleRow perf mode
contracting all 256 input channels per instruction.

Layout: per core, x is stored per (image, 8-row output block) as a padded
10-row window [c_hi 2][10 rows][1 pad col + 56 px] + 12 B zero tail
(1152 B contiguous per partition) on 128 c_lo partitions. Each conv tap is
a moving AP [c=2 (stride 570)][row=8 (stride 57)][col=56 (stride 1)] - 448
emitted columns per matmul, same monotonic stride structure as the
classic plane layout, but every window is ONE contiguous DMA run per
partition so the HW DGE moves input at full descriptor rate, and
per-block arrival semaphores gate compute exactly. Adjacent blocks
duplicate 2 halo rows (+9% input bytes, off the critical path).

Input streams on BOTH hwdge queues (scalar + sync): sync carries the
weights (tap 0-3 first so the first LDWEIGHTS waits on only 32 KB), scalar
carries x in need order: block-0 windows per image first (exactly what
sweep 0 reads), then blocks 1-6 batched over images. Warmup matmuls on
garbage bridge the PE clock-gate (HAM) ramp from the preamble barrier to
first data; the final warmup is an exact clone of the first real matmul.

Weight-stationary schedule: 14 sweeps of (block, oc) x 4 images,
oc-interleaved so each tap's weights load once and 3 matmuls reuse them;
sweep 0 and the final sweep run image-major (per-group completion spread
across the sweep instead of bunched at its end), which lets the final
sweep's output stores drain progressively - the tail after the last matmul
is only one group's cast + store, both split in half across vector/scalar
engines and sync/scalar queues. PSUM quads alternate per sweep; vector
casts (fp32 -> int32) evacuate each group as its tap 8 lands and per-group
sync-queue stores follow each cast. End-of-kernel completion relies on the
epilogue per-engine DRAIN instead of explicit semaphore waits.
"""

import numpy as np
import ml_dtypes

import concourse.bacc as bacc
import concourse.mybir as mybir
from concourse import bass_utils

N_CORES = 8
NIMG = 4          # images per core
O = 256           # out channels
H = W = 56
RB = 8            # output rows per block
NBLK = H // RB    # 7
NSW = 2 * NBLK    # 14 sweeps of (oc, block)
SEG = RB * W      # 448 output cols per group
F8 = ml_dtypes.float8_e4m3
DR = mybir.MatmulPerfMode.DoubleRow

# Per-(img, block) x window: plane rows 8b..8b+10 stored [c=2][10 rows][57]
# (row = 1 left-pad col + 56 px) plus 12 B zero tail so the tap pad-overflow
# (c=1, last row, col 57) lands in own padding. Strides c:570 > row:57 > 1
# match the proven DoubleRow rhs AP structure; each window is one 1152 B
# contiguous DMA run per partition.
WROWS = 10        # rows per block window (8 + top/bottom halo)
CSTR = WROWS * 57   # 570: c-half stride inside a window
BLKB = 2 * CSTR + 12  # 1152: window bytes per partition (12 B zero tail)
IMGS = NBLK * BLKB    # 8064 B per image per partition
X_TAIL = 128      # slack so tap_rhs's rearrange slice stays in bounds

N_WARM = 7        # warmup matmuls bridging preamble -> first data

_CACHED_NC = None


def _build_module(sim_safe=False):
    # sim_safe: skip the warmup/clone matmuls (they intentionally read
    # uninitialized SBUF, which CoreSim rejects); used only for validation.
    nc = bacc.Bacc("TRN2", target_bir_lowering=False, debug=False,
                   num_devices=N_CORES)
    # xp per partition: block-major [blk 7][img 4][1152]
    xp_d = nc.dram_tensor("xp", [128, NBLK * NIMG * BLKB], mybir.dt.float8e4,
                          kind="ExternalInput").ap()
    wt_d = nc.dram_tensor("wt", [2, 128, 9, 2, 128], mybir.dt.float8e4,
                          kind="ExternalInput").ap()
    # sweep-major output: one contiguous [NIMG, 128, 448] tile per sweep
    y_d = nc.dram_tensor("y", [2, NBLK, NIMG, 128, SEG], mybir.dt.int32,
                         kind="ExternalOutput").ap()

    w_sb = [nc.alloc_sbuf_tensor(f"w_sb{oc}", [128, 9, 2, 128],
                                 mybir.dt.float8e4).ap() for oc in range(2)]
    x_all = nc.alloc_sbuf_tensor("x_all", [128, NIMG * IMGS + X_TAIL],
                                 mybir.dt.float8e4).ap()

    ob = [nc.alloc_sbuf_tensor(f"ob{p}", [128, NIMG * SEG],
                               mybir.dt.int32).ap() for p in range(2)]
    ps = [nc.alloc_psum_tensor(f"ps{j}", [128, SEG], mybir.dt.float32).ap()
          for j in range(8)]
    # warmup operands: garbage fp8 is fine, results are discarded
    warm = nc.alloc_sbuf_tensor("warm", [128, 128 + SEG],
                                mybir.dt.float8e4).ap()

    s_w0 = nc.alloc_semaphore("s_w0")
    s_w0c = nc.alloc_semaphore("s_w0c")
    s_w1 = nc.alloc_semaphore("s_w1")
    s_xa = [nc.alloc_semaphore(f"s_xa{i}") for i in range(NIMG)]
    s_xb = [None] + [nc.alloc_semaphore(f"s_xb{b}") for b in range(1, NBLK)]
    s_mm = nc.alloc_semaphore("s_mm")      # +1 per completed group (tap 8)
    s_cast = nc.alloc_semaphore("s_cast")  # +1 per evacuated group
    s_outb = [nc.alloc_semaphore(f"s_outb{p}") for p in range(2)]
    s_lc = nc.alloc_semaphore("s_lc")      # final-sweep vector casts
    s_lcs = nc.alloc_semaphore("s_lcs")    # final-sweep scalar cast
    s_fin = nc.alloc_semaphore("s_fin")    # final-sweep store completions

    # ---- Input DMAs, all on the scalar hwdge queue in need order --------
    # w0 split so sweep 0's first LDWEIGHTS only waits for taps 0-3 (32 KB);
    # x windows: block 0 per image (critical path for sweep 0), blocks 1-6
    # batched over all 4 images in one DMA each.
    # xp layout: block-major [blk 7][img 4][1152]; x_all: [img 4][blk 7][1152]
    xi = x_all[:, 0:NIMG * IMGS].rearrange(
        "p (i b t) -> p i b t", i=NIMG, b=NBLK)
    nc.scalar.dma_start(w_sb[0][:, 0:4], wt_d[0][:, 0:4]).then_inc(s_w0, 16)
    nc.scalar.dma_start(
        xi[:, 0, 0], xp_d[:, 0:BLKB]).then_inc(s_xa[0], 16)
    nc.scalar.dma_start(w_sb[0][:, 4:9], wt_d[0][:, 4:9]).then_inc(s_w0c, 16)
    for i in range(1, NIMG):
        nc.scalar.dma_start(
            xi[:, i, 0], xp_d[:, BLKB * i:BLKB * (i + 1)]).then_inc(
            s_xa[i], 16)
    nc.scalar.dma_start(w_sb[1][:], wt_d[1]).then_inc(s_w1, 16)
    for b in range(1, NBLK):
        nc.scalar.dma_start(
            xi[:, :, b],
            xp_d[:, BLKB * NIMG * b:BLKB * NIMG * (b + 1)].rearrange(
                "p (i t) -> p i t", i=NIMG)).then_inc(s_xb[b], 16)

    # ---- Tensor engine --------------------------------------------------
    def tap_rhs(i, b, tap, rows=RB):
        dy, dx = tap // 3 - 1, tap % 3 - 1
        base = i * IMGS + b * BLKB + (1 + dy) * 57 + 1 + dx
        return x_all[:, base:base + 2 * CSTR].rearrange(
            "p (c r w) -> p c r w", c=2, w=57)[:, :, 0:rows, 0:W]

    def conv_mm(dst, oc, i, b, tap, first, last, reuse=False):
        """One conv tap: DoubleRow matmul (hardware) or two plain matmuls
        over the c halves (sim_safe; CoreSim can't model DR with 4D rhs).
        Returns the final matmul (attach then_inc to it)."""
        if not sim_safe:
            mm = nc.tensor.matmul(
                dst, lhsT=w_sb[oc][:, tap], rhs=tap_rhs(i, b, tap),
                start=first, stop=last, perf_mode=DR, skip_group_check=True)
            if reuse:
                mm.ins.ldweights = False
            return mm
        dy, dx = tap // 3 - 1, tap % 3 - 1
        for c in range(2):
            off = (i * IMGS + b * BLKB + c * CSTR
                   + (1 + dy) * 57 + 1 + dx)
            rhs = x_all[:, off:off + RB * 57].rearrange(
                "p (r w) -> p r w", w=57)[:, :, 0:W]
            mm = nc.tensor.matmul(
                dst, lhsT=w_sb[oc][:, tap, c], rhs=rhs,
                start=(first and c == 0), stop=(last and c == 1),
                skip_group_check=True)
        return mm

    # Warm the PE clock gate (HAM) with throwaway matmuls while the input
    # DMAs are in flight: the ramp to 2.4 GHz needs CONTINUOUS PE activity
    # and an idle gap resets it, so the chain bridges from the preamble
    # barrier to the first input semaphore.
    if not sim_safe:
        for _ in range(N_WARM):
            nc.tensor.matmul(ps[7][:], lhsT=warm[:, 0:128],
                             rhs=warm[:, 128:128 + SEG],
                             start=True, stop=True)
        # The final warmup is an exact clone of the first real matmul — same
        # tap-0 weights, same 4D rhs AP (reading x_all before its DMA lands:
        # garbage in, result discarded), same PSUM bank — priming weights, AP
        # walkers, and accumulator path so real work starts with no restart.
        nc.tensor.wait_ge(s_w0, 16)
        nc.tensor.matmul(
            ps[0][:, 0:W], lhsT=w_sb[0][:, 0],
            rhs=tap_rhs(0, 0, 0)[:, :, 0:1],
            start=True, stop=True, perf_mode=DR, skip_group_check=True)

    # Sweep 0 runs image-major (each image's 9 taps back to back, every
    # matmul self-loading) so compute starts as soon as image 0's top rows
    # land, while the other images' segments stream in behind it.
    nc.tensor.wait_ge(s_w0, 16)
    for i in range(NIMG):
        nc.tensor.wait_ge(s_xa[i], 16)
        for tap in range(9):
            if i == 0 and tap == 4:
                nc.tensor.wait_ge(s_w0c, 16)
            mm = conv_mm(ps[i][:], 0, i, 0, tap, tap == 0, tap == 8,
                         reuse=(i == 0 and tap == 0))
            if tap == 8:
                mm.then_inc(s_mm, 1)

    # block gate: block b's window DMA must have landed
    blk_gate = {b: (s_xb[b], 16) for b in range(1, NBLK)}
    for s in range(1, NSW - 1):
        b, oc = s // 2, s % 2
        q = 4 * (s % 2)
        if s == 1:
            nc.tensor.wait_ge(s_w1, 16)
        if oc == 0 and b in blk_gate:
            nc.tensor.wait_ge(*blk_gate[b])
        if s >= 2:
            # PSUM WAR: this quad was last used by sweep s-2
            nc.tensor.wait_ge(s_cast, 4 * s - 4)
        for tap in range(9):
            for i in range(NIMG):
                # i > 0 reuses the weights the i == 0 matmul loaded
                mm = conv_mm(ps[q + i][:], oc, i, b, tap,
                             tap == 0, tap == 8, reuse=(i > 0))
                if tap == 8:
                    mm.then_inc(s_mm, 1)

    # Final sweep image-major: group i completes 9/36ths of a sweep after
    # group i-1, so casts + stores drain progressively and the post-matmul
    # tail is a single group's evacuation.
    sl = NSW - 1
    nc.tensor.wait_ge(s_cast, 4 * sl - 4)
    for i in range(NIMG):
        for tap in range(9):
            mm = conv_mm(ps[4 + i][:], 1, i, 6, tap, tap == 0, tap == 8)
            if tap == 8:
                mm.then_inc(s_mm, 1)

    # ---- Vector engine: PSUM -> int32 SBUF staging ----------------------
    def cast_group(eng, s, i, sem, inc, cols=slice(None)):
        q = 4 * (s % 2)
        op = getattr(eng, "tensor_copy", None) or eng.copy
        dst = ob[s % 2].rearrange("p (i t) -> p i t", i=NIMG)[:, i, cols]
        op(dst, ps[q + i][:, cols]).then_inc(sem, inc)

    for s in range(NSW - 1):
        for i in range(NIMG):
            if s >= 2 and i == 0:
                # ob WAR: all four of sweep s-2's stores have read ob
                # (coarse on purpose: store completion order within the
                # sync queue is FIFO but not semaphore-distinguishable)
                nc.vector.wait_ge(s_outb[s % 2], 64 * (s // 2))
            nc.vector.wait_ge(s_mm, 4 * s + i + 1)
            cast_group(nc.vector, s, i, s_cast, 1)

    # ---- Sync engine: per-group output stores ---------------------------
    # one store per (sweep, image), issued as soon as that cast lands
    obl = [ob[p].rearrange("p (i t) -> p i t", i=NIMG) for p in range(2)]
    for s in range(NSW - 1):
        b, oc = s // 2, s % 2
        for i in range(NIMG):
            nc.sync.wait_ge(s_cast, 4 * s + i + 1)
            nc.sync.dma_start(y_d[oc, b, i], obl[s % 2][:, i]).then_inc(
                s_outb[s % 2], 16)

    # Final sweep (image-major): groups complete 9 matmuls apart, so each
    # cast + store drains while the next group computes. Casts split
    # vector (i0, i3) / scalar (i1, i2); stores sync (i0, i3) / scalar.
    war = 64 * (sl // 2)
    nc.vector.wait_ge(s_outb[1], war)
    nc.vector.wait_ge(s_mm, 4 * sl + 1)
    cast_group(nc.vector, sl, 0, s_lc, 1)
    nc.sync.wait_ge(s_lc, 1)
    nc.sync.dma_start(y_d[1, 6, 0], obl[1][:, 0]).then_inc(s_fin, 16)
    nc.scalar.wait_ge(s_outb[1], war)
    nc.scalar.wait_ge(s_mm, 4 * sl + 2)
    cast_group(nc.scalar, sl, 1, s_lcs, 1)
    nc.scalar.wait_ge(s_lcs, 1)
    nc.scalar.dma_start(y_d[1, 6, 1], obl[1][:, 1]).then_inc(s_fin, 16)
    nc.scalar.wait_ge(s_mm, 4 * sl + 3)
    cast_group(nc.scalar, sl, 2, s_lcs, 1)
    nc.scalar.wait_ge(s_lcs, 2)
    nc.scalar.dma_start(y_d[1, 6, 2], obl[1][:, 2]).then_inc(s_fin, 16)
    nc.vector.wait_ge(s_mm, 4 * sl + 4)
    cast_group(nc.vector, sl, 3, s_lc, 1)
    nc.sync.wait_ge(s_lc, 2)
    nc.sync.dma_start(y_d[1, 6, 3], obl[1][:, 3]).then_inc(s_fin, 16)

    # No explicit waits on the output-DMA completion here: the per-engine
    # DRAIN in the epilogue barrier blocks until each queue's DMAs finish.
    nc.sync.drain()
    nc.all_engine_barrier()
    nc.compile()
    return nc


def _get_nc():
    global _CACHED_NC
    if _CACHED_NC is None:
        _CACHED_NC = _build_module()
    return _CACHED_NC


def _prep_inputs(x: np.ndarray, weight: np.ndarray):
    """Host-side conversion to the kernel's DRAM layouts (exact for the
    quantized value ranges)."""
    xr = x.astype(np.float32).astype(F8).reshape(N_CORES, NIMG, 2, 128, H, W)
    # padded plane per (core, part, img, c): rows 0..57 with halos, 57 cols
    pad = np.zeros((N_CORES, 128, NIMG, 2, 58, 57), F8)
    pad[:, :, :, :, 1:H + 1, 1:W + 1] = xr.transpose(0, 3, 1, 2, 4, 5)
    # block windows: [core, part, blk, img, c, 10, 57] + 12 B zero tail
    xp_all = np.zeros((N_CORES, 128, NBLK, NIMG, BLKB), F8)
    for b in range(NBLK):
        win = pad[:, :, :, :, RB * b:RB * b + WROWS]  # [.., img, c, 10, 57]
        xp_all[:, :, b, :, :2 * CSTR] = win.reshape(
            N_CORES, 128, NIMG, 2 * CSTR)
    xp_all = xp_all.reshape(N_CORES, 128, NBLK * NIMG * BLKB)

    wt = weight.astype(np.float32).astype(F8)
    # (O, C, 3, 3) -> [oc][c_lo][tap][c_hi][o_in_half]
    wt = wt.reshape(2, 128, 2, 128, 3, 3).transpose(0, 3, 4, 5, 2, 1)
    wt2 = np.ascontiguousarray(wt.reshape(2, 128, 9, 2, 128))
    return xp_all, wt2


def run_on_device(x: np.ndarray, weight: np.ndarray, **run_kwargs):
    """Build in_maps, run the SPMD kernel on 8 cores, return (y, results)."""
    nc = _get_nc()
    xp_all, wt2 = _prep_inputs(x, weight)
    in_maps = [{"xp": xp_all[c], "wt": wt2} for c in range(N_CORES)]
    res = bass_utils.run_bass_kernel_spmd(
        nc, in_maps, core_ids=list(range(N_CORES)), **run_kwargs)
    y = np.concatenate(
        [res.results[c]["y"].reshape(2, NBLK, NIMG, 128, RB, W)
         .transpose(2, 0, 3, 1, 4, 5).reshape(NIMG, O, H, W)
         for c in range(N_CORES)], axis=0)
    return y, res


def kernel(x: np.ndarray, weight: np.ndarray) -> np.ndarray:
    y, _ = run_on_device(np.asarray(x), np.asarray(weight))
    return y
